# revision 4
# baseline (speedup 1.0000x reference)
"""FBPINN (windowed mixture of per-subdomain MLPs) Trainium2 kernel.

Strategy: the cosine partition-of-unity window has compact support — a
point contributes to a subdomain's MLP only if it lies strictly inside
that subdomain's box.  With the 8x4 overlapped tiling each point lands in
~2.5 of the 32 boxes, so dense evaluation wastes >90% of the FLOPs.

  host:   route points -> per-expert gathered (padded) point lists
  device: 8 cores x 4 experts each; experts packed in pairs into the
          128-partition systolic array (block-diagonal weights); tiny
          MLP in feature-major layout, tanh+bias fused on the ACT engine
          reading PSUM directly.
  host:   scatter-add  w*(o+bo)  and  w  per point, final normalize.

All heavy FLOPs (the 4 matmul layers + tanh) run on device in fp32.
"""

import numpy as np

import concourse.bacc as bacc
import concourse.mybir as mybir
import concourse.tile as tile
from concourse.bass_utils import run_bass_kernel_spmd

# problem constants (hardcoded per contract)
N_PTS = 32768
S = 32
XDIM = 2
WIDTH = 64
TRANS = 0.1
TOL = 1e-8
N_CORES = 8
E_PER_CORE = S // N_CORES      # 4 experts per core
PAIRS = E_PER_CORE // 2        # 2 block-diag pairs per core

MM_CH = 512                    # fp32 moving-operand max per matmul
ACT_CH = 1024                  # ACT reads 2 psum banks per instruction

_compiled_cache: dict[int, object] = {}


def _build_nc(p_pad: int):
    """Bass program: PAIRS block-diag expert pairs, points padded to p_pad."""
    fp32 = mybir.dt.float32
    nc = bacc.Bacc("TRN2", target_bir_lowering=False, debug=False,
                   num_devices=N_CORES)

    xt_d = nc.dram_tensor("xt", [PAIRS, 4, p_pad], fp32, kind="ExternalInput")
    w0_d = nc.dram_tensor("w0", [PAIRS, 4, 128], fp32, kind="ExternalInput")
    w1_d = nc.dram_tensor("w1", [PAIRS, 128, 128], fp32, kind="ExternalInput")
    w2_d = nc.dram_tensor("w2", [PAIRS, 128, 128], fp32, kind="ExternalInput")
    wo_d = nc.dram_tensor("wo", [PAIRS, 128, 2], fp32, kind="ExternalInput")
    b0_d = nc.dram_tensor("b0", [PAIRS, 128, 1], fp32, kind="ExternalInput")
    b1_d = nc.dram_tensor("b1", [PAIRS, 128, 1], fp32, kind="ExternalInput")
    b2_d = nc.dram_tensor("b2", [PAIRS, 128, 1], fp32, kind="ExternalInput")
    oo_d = nc.dram_tensor("oo", [PAIRS, 2, p_pad], fp32, kind="ExternalOutput")

    tanh = mybir.ActivationFunctionType.Tanh
    n_act = p_pad // ACT_CH

    with tile.TileContext(nc) as tc:
        with (
            tc.tile_pool(name="wpool", bufs=2) as wpool,
            tc.tile_pool(name="hpool", bufs=2) as hpool,
            tc.tile_pool(name="ps", bufs=1, space="PSUM") as ps,
            tc.tile_pool(name="pso", bufs=2, space="PSUM") as psop,
        ):
            for p in range(PAIRS):
                xt = wpool.tile([4, p_pad], fp32, tag="xt")
                w0 = wpool.tile([4, 128], fp32, tag="w0")
                w1 = wpool.tile([128, 128], fp32, tag="w1")
                w2 = wpool.tile([128, 128], fp32, tag="w2")
                wo = wpool.tile([128, 2], fp32, tag="wo")
                b0 = wpool.tile([128, 1], fp32, tag="b0")
                b1 = wpool.tile([128, 1], fp32, tag="b1")
                b2 = wpool.tile([128, 1], fp32, tag="b2")
                nc.sync.dma_start(xt[:], xt_d[p])
                nc.sync.dma_start(w0[:], w0_d[p])
                nc.sync.dma_start(w1[:], w1_d[p])
                nc.sync.dma_start(w2[:], w2_d[p])
                nc.sync.dma_start(wo[:], wo_d[p])
                nc.sync.dma_start(b0[:], b0_d[p])
                nc.sync.dma_start(b1[:], b1_d[p])
                nc.sync.dma_start(b2[:], b2_d[p])

                h0 = hpool.tile([128, p_pad], fp32, tag="h0")
                h1 = hpool.tile([128, p_pad], fp32, tag="h1")
                h2 = hpool.tile([128, p_pad], fp32, tag="h2")
                o_sb = hpool.tile([2, p_pad], fp32, tag="o_sb")

                for c in range(n_act):
                    a0 = c * ACT_CH
                    ps0 = ps.tile([128, ACT_CH], fp32, tag="ps0")
                    for m in range(ACT_CH // MM_CH):
                        o = m * MM_CH
                        nc.tensor.matmul(ps0[:, o:o + MM_CH], w0[:],
                                         xt[:, a0 + o:a0 + o + MM_CH],
                                         start=True, stop=True)
                    nc.scalar.activation(h0[:, a0:a0 + ACT_CH], ps0[:],
                                         tanh, bias=b0[:])

                    ps1 = ps.tile([128, ACT_CH], fp32, tag="ps1")
                    for m in range(ACT_CH // MM_CH):
                        o = m * MM_CH
                        nc.tensor.matmul(ps1[:, o:o + MM_CH], w1[:],
                                         h0[:, a0 + o:a0 + o + MM_CH],
                                         start=True, stop=True)
                    nc.scalar.activation(h1[:, a0:a0 + ACT_CH], ps1[:],
                                         tanh, bias=b1[:])

                    ps2 = ps.tile([128, ACT_CH], fp32, tag="ps2")
                    for m in range(ACT_CH // MM_CH):
                        o = m * MM_CH
                        nc.tensor.matmul(ps2[:, o:o + MM_CH], w2[:],
                                         h1[:, a0 + o:a0 + o + MM_CH],
                                         start=True, stop=True)
                    nc.scalar.activation(h2[:, a0:a0 + ACT_CH], ps2[:],
                                         tanh, bias=b2[:])

                    for m in range(ACT_CH // MM_CH):
                        o = a0 + m * MM_CH
                        pso = psop.tile([2, MM_CH], fp32, tag="pso")
                        nc.tensor.matmul(pso[:], wo[:], h2[:, o:o + MM_CH],
                                         start=True, stop=True)
                        nc.vector.tensor_copy(o_sb[:, o:o + MM_CH], pso[:])
                nc.sync.dma_start(oo_d[p], o_sb[:])
    nc.compile()
    return nc


def _get_nc(p_pad: int):
    nc = _compiled_cache.get(p_pad)
    if nc is None:
        nc = _build_nc(p_pad)
        _compiled_cache[p_pad] = nc
    return nc


def kernel(x, xmins, xmaxs, W0, b0, W1, b1, W2, b2, Wo, bo):
    x = np.asarray(x)
    n_pts = x.shape[0]
    xmins64 = np.asarray(xmins, np.float64)
    xmaxs64 = np.asarray(xmaxs, np.float64)
    x64 = np.asarray(x, np.float64)

    # ---- host routing: strict-interior membership == window support ----
    inside = ((x[:, None, :] > xmins[None, :, :])
              & (x[:, None, :] < xmaxs[None, :, :])).all(-1)      # (N, S)
    idx = [np.nonzero(inside[:, s])[0] for s in range(S)]
    counts = np.array([len(i) for i in idx])
    p_pad = int(max(ACT_CH, -(-int(counts.max()) // ACT_CH) * ACT_CH))

    # ---- window values (float64, exact same formula as reference) ----
    # computed sparsely per expert below, plus dense denominator
    def window_vals(pts64, s):
        tu = np.clip((pts64 - xmins64[s]) / TRANS, 0.0, 1.0)
        td = np.clip((xmaxs64[s] - pts64) / TRANS, 0.0, 1.0)
        per = 0.25 * (1.0 - np.cos(np.pi * tu)) * (1.0 - np.cos(np.pi * td))
        return per.prod(-1)

    # ---- fold input normalization into layer-0 weights (float64) ----
    center = 0.5 * (xmins64 + xmaxs64)                            # (S, 2)
    scale = np.maximum(0.5 * (xmaxs64 - xmins64), 1e-9)
    W0f = np.asarray(W0, np.float64) / scale[:, None, :]          # (S, 64, 2)
    b0f = np.asarray(b0, np.float64) - (W0f * center[:, None, :]).sum(-1)

    # ---- pack per-core device inputs ----
    W1 = np.asarray(W1)
    W2 = np.asarray(W2)
    Wo = np.asarray(Wo)
    b1 = np.asarray(b1)
    b2 = np.asarray(b2)
    in_maps = []
    for core in range(N_CORES):
        xt = np.zeros((PAIRS, 4, p_pad), np.float32)
        w0p = np.zeros((PAIRS, 4, 128), np.float32)
        w1p = np.zeros((PAIRS, 128, 128), np.float32)
        w2p = np.zeros((PAIRS, 128, 128), np.float32)
        wop = np.zeros((PAIRS, 128, 2), np.float32)
        b0p = np.zeros((PAIRS, 128, 1), np.float32)
        b1p = np.zeros((PAIRS, 128, 1), np.float32)
        b2p = np.zeros((PAIRS, 128, 1), np.float32)
        for p in range(PAIRS):
            for j in range(2):
                s = core * E_PER_CORE + 2 * p + j
                lo, hi = 64 * j, 64 * (j + 1)
                pts = x[idx[s]]                                   # (P_s, 2)
                xt[p, 2 * j:2 * j + 2, :len(pts)] = pts.T
                w0p[p, 2 * j:2 * j + 2, lo:hi] = W0f[s].T
                w1p[p, lo:hi, lo:hi] = W1[s].T
                w2p[p, lo:hi, lo:hi] = W2[s].T
                wop[p, lo:hi, j] = Wo[s, 0, :]
                b0p[p, lo:hi, 0] = b0f[s]
                b1p[p, lo:hi, 0] = b1[s]
                b2p[p, lo:hi, 0] = b2[s]
        in_maps.append({"xt": xt, "w0": w0p, "w1": w1p, "w2": w2p,
                        "wo": wop, "b0": b0p, "b1": b1p, "b2": b2p})

    # ---- run on 8 cores ----
    global _last_in_maps
    _last_in_maps = in_maps
    nc = _get_nc(p_pad)
    res = run_bass_kernel_spmd(nc, in_maps, core_ids=list(range(N_CORES)),
                               trace=False)

    # ---- host scatter-add + normalize ----
    num = np.zeros(n_pts, np.float64)
    den = np.zeros(n_pts, np.float64)
    bo = np.asarray(bo, np.float64)
    for core in range(N_CORES):
        oo = res.results[core]["oo"]                              # (PAIRS, 2, p_pad)
        for p in range(PAIRS):
            for j in range(2):
                s = core * E_PER_CORE + 2 * p + j
                ii = idx[s]
                if len(ii) == 0:
                    continue
                w = window_vals(x64[ii], s)                       # (P_s,)
                num[ii] += w * (oo[p, j, :len(ii)].astype(np.float64) + bo[s, 0])
                den[ii] += w
    y = num / (den + TOL)
    return y.astype(np.float32).reshape(n_pts, 1)


# revision 5
# speedup vs baseline: 1.0175x; 1.0175x over previous
"""FBPINN (windowed mixture of per-subdomain MLPs) Trainium2 kernel.

Strategy: the cosine partition-of-unity window has compact support — a
point contributes to a subdomain's MLP only if it lies strictly inside
that subdomain's box.  With the 8x4 overlapped tiling each point lands in
~2.5 of the 32 boxes, so dense evaluation wastes >90% of the FLOPs.

  host:   route points -> per-expert gathered (padded) point lists
  device: 8 cores x 4 experts each; experts packed in pairs into the
          128-partition systolic array (block-diagonal weights); tiny
          MLP in feature-major layout, tanh+bias fused on the ACT engine
          reading PSUM directly.
  host:   scatter-add  w*(o+bo)  and  w  per point, final normalize.

All heavy FLOPs (the 4 matmul layers + tanh) run on device in fp32.
"""

import numpy as np

import concourse.bacc as bacc
import concourse.mybir as mybir
import concourse.tile as tile
from concourse.bass_utils import run_bass_kernel_spmd

# problem constants (hardcoded per contract)
N_PTS = 32768
S = 32
XDIM = 2
WIDTH = 64
TRANS = 0.1
TOL = 1e-8
N_CORES = 8
E_PER_CORE = S // N_CORES      # 4 experts per core
PAIRS = E_PER_CORE // 2        # 2 block-diag pairs per core

MM_CH = 512                    # fp32 moving-operand max per matmul
ACT_CH = 1024                  # ACT reads 2 psum banks per instruction

_compiled_cache: dict[int, object] = {}


def _build_nc(p_pad: int):
    """Bass program: PAIRS block-diag expert pairs, points padded to p_pad.

    Matmul operands use float32r (single-pass PE streaming: 4x the fp32
    rate, ~1.6e-4 relative precision measured on HW); PSUM accumulation
    stays fp32.
    """
    fp32 = mybir.dt.float32
    fp32r = mybir.dt.float32r
    nc = bacc.Bacc("TRN2", target_bir_lowering=False, debug=False,
                   num_devices=N_CORES)

    xt_d = nc.dram_tensor("xt", [PAIRS, 4, p_pad], fp32r, kind="ExternalInput")
    w0_d = nc.dram_tensor("w0", [PAIRS, 4, 128], fp32r, kind="ExternalInput")
    w1_d = nc.dram_tensor("w1", [PAIRS, 128, 128], fp32r, kind="ExternalInput")
    w2_d = nc.dram_tensor("w2", [PAIRS, 128, 128], fp32r, kind="ExternalInput")
    wo_d = nc.dram_tensor("wo", [PAIRS, 128, 2], fp32r, kind="ExternalInput")
    b0_d = nc.dram_tensor("b0", [PAIRS, 128, 1], fp32, kind="ExternalInput")
    b1_d = nc.dram_tensor("b1", [PAIRS, 128, 1], fp32, kind="ExternalInput")
    b2_d = nc.dram_tensor("b2", [PAIRS, 128, 1], fp32, kind="ExternalInput")
    oo_d = nc.dram_tensor("oo", [PAIRS, 2, p_pad], fp32, kind="ExternalOutput")

    tanh = mybir.ActivationFunctionType.Tanh
    n_act = p_pad // ACT_CH

    with tile.TileContext(nc) as tc:
        with (
            tc.tile_pool(name="wpool", bufs=2) as wpool,
            tc.tile_pool(name="hpool", bufs=2) as hpool,
            tc.tile_pool(name="ps", bufs=1, space="PSUM") as ps,
            tc.tile_pool(name="pso", bufs=2, space="PSUM") as psop,
        ):
            for p in range(PAIRS):
                xt = wpool.tile([4, p_pad], fp32r, tag="xt")
                w0 = wpool.tile([4, 128], fp32r, tag="w0")
                w1 = wpool.tile([128, 128], fp32r, tag="w1")
                w2 = wpool.tile([128, 128], fp32r, tag="w2")
                wo = wpool.tile([128, 2], fp32r, tag="wo")
                b0 = wpool.tile([128, 1], fp32, tag="b0")
                b1 = wpool.tile([128, 1], fp32, tag="b1")
                b2 = wpool.tile([128, 1], fp32, tag="b2")
                nc.sync.dma_start(xt[:], xt_d[p])
                nc.sync.dma_start(w0[:], w0_d[p])
                nc.sync.dma_start(w1[:], w1_d[p])
                nc.sync.dma_start(w2[:], w2_d[p])
                nc.sync.dma_start(wo[:], wo_d[p])
                nc.sync.dma_start(b0[:], b0_d[p])
                nc.sync.dma_start(b1[:], b1_d[p])
                nc.sync.dma_start(b2[:], b2_d[p])

                h0 = hpool.tile([128, p_pad], fp32r, tag="h0")
                h1 = hpool.tile([128, p_pad], fp32r, tag="h1")
                h2 = hpool.tile([128, p_pad], fp32r, tag="h2")
                o_sb = hpool.tile([2, p_pad], fp32, tag="o_sb")

                for c in range(n_act):
                    a0 = c * ACT_CH
                    ps0 = ps.tile([128, ACT_CH], fp32, tag="ps0")
                    for m in range(ACT_CH // MM_CH):
                        o = m * MM_CH
                        nc.tensor.matmul(ps0[:, o:o + MM_CH], w0[:],
                                         xt[:, a0 + o:a0 + o + MM_CH],
                                         start=True, stop=True)
                    nc.scalar.activation(h0[:, a0:a0 + ACT_CH], ps0[:],
                                         tanh, bias=b0[:])

                    ps1 = ps.tile([128, ACT_CH], fp32, tag="ps1")
                    for m in range(ACT_CH // MM_CH):
                        o = m * MM_CH
                        nc.tensor.matmul(ps1[:, o:o + MM_CH], w1[:],
                                         h0[:, a0 + o:a0 + o + MM_CH],
                                         start=True, stop=True)
                    nc.scalar.activation(h1[:, a0:a0 + ACT_CH], ps1[:],
                                         tanh, bias=b1[:])

                    ps2 = ps.tile([128, ACT_CH], fp32, tag="ps2")
                    for m in range(ACT_CH // MM_CH):
                        o = m * MM_CH
                        nc.tensor.matmul(ps2[:, o:o + MM_CH], w2[:],
                                         h1[:, a0 + o:a0 + o + MM_CH],
                                         start=True, stop=True)
                    nc.scalar.activation(h2[:, a0:a0 + ACT_CH], ps2[:],
                                         tanh, bias=b2[:])

                    for m in range(ACT_CH // MM_CH):
                        o = a0 + m * MM_CH
                        pso = psop.tile([2, MM_CH], fp32, tag="pso")
                        nc.tensor.matmul(pso[:], wo[:], h2[:, o:o + MM_CH],
                                         start=True, stop=True)
                        nc.vector.tensor_copy(o_sb[:, o:o + MM_CH], pso[:])
                nc.sync.dma_start(oo_d[p], o_sb[:])
    nc.compile()
    return nc


def _get_nc(p_pad: int):
    nc = _compiled_cache.get(p_pad)
    if nc is None:
        nc = _build_nc(p_pad)
        _compiled_cache[p_pad] = nc
    return nc


def kernel(x, xmins, xmaxs, W0, b0, W1, b1, W2, b2, Wo, bo):
    x = np.asarray(x)
    n_pts = x.shape[0]
    xmins64 = np.asarray(xmins, np.float64)
    xmaxs64 = np.asarray(xmaxs, np.float64)
    x64 = np.asarray(x, np.float64)

    # ---- host routing: strict-interior membership == window support ----
    inside = ((x[:, None, :] > xmins[None, :, :])
              & (x[:, None, :] < xmaxs[None, :, :])).all(-1)      # (N, S)
    idx = [np.nonzero(inside[:, s])[0] for s in range(S)]
    counts = np.array([len(i) for i in idx])
    p_pad = int(max(ACT_CH, -(-int(counts.max()) // ACT_CH) * ACT_CH))

    # ---- window values (float64, exact same formula as reference) ----
    # computed sparsely per expert below, plus dense denominator
    def window_vals(pts64, s):
        tu = np.clip((pts64 - xmins64[s]) / TRANS, 0.0, 1.0)
        td = np.clip((xmaxs64[s] - pts64) / TRANS, 0.0, 1.0)
        per = 0.25 * (1.0 - np.cos(np.pi * tu)) * (1.0 - np.cos(np.pi * td))
        return per.prod(-1)

    # ---- fold input normalization into layer-0 weights (float64) ----
    center = 0.5 * (xmins64 + xmaxs64)                            # (S, 2)
    scale = np.maximum(0.5 * (xmaxs64 - xmins64), 1e-9)
    W0f = np.asarray(W0, np.float64) / scale[:, None, :]          # (S, 64, 2)
    b0f = np.asarray(b0, np.float64) - (W0f * center[:, None, :]).sum(-1)

    # ---- pack per-core device inputs ----
    W1 = np.asarray(W1)
    W2 = np.asarray(W2)
    Wo = np.asarray(Wo)
    b1 = np.asarray(b1)
    b2 = np.asarray(b2)
    in_maps = []
    for core in range(N_CORES):
        xt = np.zeros((PAIRS, 4, p_pad), np.float32)
        w0p = np.zeros((PAIRS, 4, 128), np.float32)
        w1p = np.zeros((PAIRS, 128, 128), np.float32)
        w2p = np.zeros((PAIRS, 128, 128), np.float32)
        wop = np.zeros((PAIRS, 128, 2), np.float32)
        b0p = np.zeros((PAIRS, 128, 1), np.float32)
        b1p = np.zeros((PAIRS, 128, 1), np.float32)
        b2p = np.zeros((PAIRS, 128, 1), np.float32)
        for p in range(PAIRS):
            for j in range(2):
                s = core * E_PER_CORE + 2 * p + j
                lo, hi = 64 * j, 64 * (j + 1)
                pts = x[idx[s]]                                   # (P_s, 2)
                xt[p, 2 * j:2 * j + 2, :len(pts)] = pts.T
                w0p[p, 2 * j:2 * j + 2, lo:hi] = W0f[s].T
                w1p[p, lo:hi, lo:hi] = W1[s].T
                w2p[p, lo:hi, lo:hi] = W2[s].T
                wop[p, lo:hi, j] = Wo[s, 0, :]
                b0p[p, lo:hi, 0] = b0f[s]
                b1p[p, lo:hi, 0] = b1[s]
                b2p[p, lo:hi, 0] = b2[s]
        in_maps.append({"xt": xt, "w0": w0p, "w1": w1p, "w2": w2p,
                        "wo": wop, "b0": b0p, "b1": b1p, "b2": b2p})

    # ---- run on 8 cores ----
    global _last_in_maps
    _last_in_maps = in_maps
    nc = _get_nc(p_pad)
    res = run_bass_kernel_spmd(nc, in_maps, core_ids=list(range(N_CORES)),
                               trace=False)

    # ---- host scatter-add + normalize ----
    num = np.zeros(n_pts, np.float64)
    den = np.zeros(n_pts, np.float64)
    bo = np.asarray(bo, np.float64)
    for core in range(N_CORES):
        oo = res.results[core]["oo"]                              # (PAIRS, 2, p_pad)
        for p in range(PAIRS):
            for j in range(2):
                s = core * E_PER_CORE + 2 * p + j
                ii = idx[s]
                if len(ii) == 0:
                    continue
                w = window_vals(x64[ii], s)                       # (P_s,)
                num[ii] += w * (oo[p, j, :len(ii)].astype(np.float64) + bo[s, 0])
                den[ii] += w
    y = num / (den + TOL)
    return y.astype(np.float32).reshape(n_pts, 1)
